# revision 14
# baseline (speedup 1.0000x reference)
"""Trainium2 Bass kernel for nn_Decoder_MDCBlock1 (MDCBlock1 decoder fusion).

Data-parallel over batch: 8 samples -> 8 NeuronCores, one sample each.
Per core: 12 dynamic convs (6 down / 6 up) with per-sample synthesized weights
W_b = w + sum_s att[b,s] * (t_s o m_s), computed on device.

Layouts (per core, SBUF), prow/pcol = img coord + 1 (zero-padded border):
  F   : [32,128,128] fusion tensor, spatial-split-4: [(q*32+ci), 36*130]
        partition q*32+ci holds padded rows 32q..32q+35.
  Y0  : [64,64,64] d0-out, spatial-split-2: [(q*64+ci), 36*66]
  Y1  : [128,32,32] padded plain: [128, 34*34]
  Y2  : [256,16,16] padded, ch-blocked: [128, 2*(18*18)]
  U2o : like Y1
  Ta  : [64,64,64] up0-input, b-packed: [(b*64+ci), 66*66]; block1 = cols+1
  f3  : final out [128=(yq*32+ci), 32*128]  (plain f32)
Weights (fp32r):
  W_T0: [(q*32+ci), 16*64]   (replicated x4 over q); W_T1: [(q*64+ci),16*128] (x2)
  W_T2: [128, 16*256]; W_U2: [128, 2*(128*16)]; W_U1: [128, 64*16]
  W_U0: [(b*64+ci), 32*16]   block1 content shifted by +2 in kk
Down conv: out[co,y,x] = sum W[co,ci,ky,kx]*xp[ci,2y+ky,2x+kx]
Up conv  : out[co,2J+py,2X+px] = sum_{a,b} W[ci,co,3-py-2a,3-px-2b]*xp[ci,J+py+a,X+px+b]
"""
import sys

if "/opt/trn_rl_repo" not in sys.path:
    sys.path.insert(0, "/opt/trn_rl_repo")

import numpy as np

NCORES = 8

_BUILT = {}


def _build(debug=False):
    import concourse.mybir as mybir
    from concourse import bacc
    from concourse.tile import TileContext
    from concourse.masks import make_identity

    f32 = mybir.dt.float32
    f32r = mybir.dt.float32r
    AF = mybir.ActivationFunctionType
    OP = mybir.AluOpType

    nc = bacc.Bacc("TRN2", target_bir_lowering=False, debug=False)

    # ---------------- DRAM I/O ----------------
    x_d = nc.dram_tensor("x", [32, 128 * 128], f32, kind="ExternalInput").ap()
    l0_d = nc.dram_tensor("l0", [256, 256], f32, kind="ExternalInput").ap()
    l1_d = nc.dram_tensor("l1", [128, 1024], f32, kind="ExternalInput").ap()
    l2_d = nc.dram_tensor("l2", [64, 4096], f32, kind="ExternalInput").ap()
    att_d = nc.dram_tensor("att", [1, 4], f32, kind="ExternalInput").ap()

    # set dims: A = leading dim of stored weight, B = trailing channel dim
    # down j: A=cout, B=cin ; up j: A=cin_up, B=cout_up.  bias: down A, up B.
    SETS = {}
    for j, name in enumerate(["d0", "d1", "d2"]):
        SETS[name] = dict(A=64 * 2**j, B=32 * 2**j, bias=64 * 2**j)
    for j, name in enumerate(["u0", "u1", "u2"]):
        SETS[name] = dict(A=64 * 2**j, B=32 * 2**j, bias=32 * 2**j)
    P = {}
    for s, d in SETS.items():
        A, B, nb = d["A"], d["B"], d["bias"]
        P[s] = dict(
            w=nc.dram_tensor(f"{s}_w", [A, B * 16], f32, kind="ExternalInput").ap(),
            ts=[nc.dram_tensor(f"{s}_t{k}", [A, B * 16], f32, kind="ExternalInput").ap()
                for k in range(4)],
            ms=[nc.dram_tensor(f"{s}_m{k}", [A, B], f32, kind="ExternalInput").ap()
                for k in range(4)],
            b=nc.dram_tensor(f"{s}_b", [nb, 1], f32, kind="ExternalInput").ap(),
            a=nc.dram_tensor(f"{s}_a", [1, 1], f32, kind="ExternalInput").ap(),
        )

    out_d = nc.dram_tensor("out", [32, 128 * 128], f32, kind="ExternalOutput").ap()
    dbg = {}
    if debug:
        for nm, shape in [("dY0", [128, 36 * 66]), ("dY1", [128, 34 * 34]),
                          ("dY2", [128, 2 * 324]), ("dU2o", [128, 34 * 34]),
                          ("dTa", [128, 66 * 66]), ("dF", [128, 36 * 130])]:
            dbg[nm] = nc.dram_tensor(nm, shape, f32r, kind="ExternalOutput").ap()

    # ---------------- static SBUF ----------------
    def sb(name, shape, dt):
        return nc.alloc_sbuf_tensor(name, list(shape), dt).ap()

    F = sb("F", [128, 36 * 130], f32r)
    f3 = sb("f3", [128, 32 * 128], f32)
    Y0 = sb("Y0", [128, 36 * 66], f32r)
    Y1 = sb("Y1", [128, 34 * 34], f32r)
    Y2 = sb("Y2", [128, 2 * 324], f32r)
    U2o = sb("U2o", [128, 34 * 34], f32r)
    Ta = sb("Ta", [128, 66 * 66], f32r)
    W_T0 = sb("W_T0", [128, 16 * 64], f32r)
    W_T1 = sb("W_T1", [128, 16 * 128], f32r)
    W_T2 = sb("W_T2", [128, 16 * 256], f32r)
    W_U2 = sb("W_U2", [128, 2 * 128 * 16], f32r)
    W_U1 = sb("W_U1", [128, 64 * 16], f32r)
    W_U0 = sb("W_U0", [128, 32 * 36], f32r)  # padded 6x6 kernel positions
    l0_t = sb("l0_t", [128, 2 * 256], f32)
    l1_t = sb("l1_t", [128, 1024], f32)
    l2_t = sb("l2_t", [128, 2048], f32)
    att_b = sb("att_b", [128, 4], f32)
    ident = sb("ident", [128, 128], f32)
    bias_t = {s: sb(f"bias_{s}", [128, 1], f32) for s in SETS}
    alpha_t = {s: sb(f"alpha_{s}", [128, 1], f32) for s in SETS}
    bias2_d2 = sb("bias2_d2", [128, 1], f32)

    W3 = {
        "F": F.rearrange("p (r c) -> p r c", c=130),
        "Y0": Y0.rearrange("p (r c) -> p r c", c=66),
        "Y1": Y1.rearrange("p (r c) -> p r c", c=34),
        "Y2": Y2.rearrange("p (cb r c) -> p cb r c", cb=2, c=18),
        "U2o": U2o.rearrange("p (r c) -> p r c", c=34),
        "Ta": Ta.rearrange("p (r c) -> p r c", c=66),
        "f3": f3.rearrange("p (r c) -> p r c", c=128),
    }

    with TileContext(nc) as tc:
        with tc.tile_pool(name="stage", bufs=3) as stage_pool, \
             tc.tile_pool(name="mstage", bufs=2) as mstage_pool, \
             tc.tile_pool(name="tmp", bufs=2) as tmp_pool, \
             tc.tile_pool(name="syn", bufs=2) as syn_pool, \
             tc.tile_pool(name="epi", bufs=3) as epi_pool, \
             tc.tile_pool(name="cpsum", bufs=6, space="PSUM") as cpsum_pool, \
             tc.tile_pool(name="tpsum", bufs=2, space="PSUM") as tpsum_pool:

            # ---------- one-time setup ----------
            zeros = sb("zeros", [128, 1], f32)
            nc.vector.memset(zeros[:, :], 0.0)

            def zfill(dst, eng=None):
                Pn = dst.shape[0]
                zin = zeros[0:Pn, 0:1]
                while zin.ndim < dst.ndim:
                    zin = zin.unsqueeze(zin.ndim)
                (eng or nc.gpsimd).tensor_copy(out=dst, in_=zin.broadcast_to(dst.shape))

            # zero pad borders (tiles are f32r; memset can't write f32r)
            zfill(W3["F"][:, :, 0:130:129])
            zfill(W3["F"][0:32, 0:1, :])
            zfill(W3["F"][96:128, 33:36, :])
            zfill(W3["Y0"][:, :, 0:66:65], nc.vector)
            zfill(W3["Y0"][0:64, 0:1, :], nc.vector)
            zfill(W3["Y0"][64:128, 33:36, :], nc.vector)
            zfill(W3["Y1"][:, 0:34:33, :], nc.vector)
            zfill(W3["Y1"][:, :, 0:34:33], nc.vector)
            for cb in range(2):
                zfill(W3["Y2"][:, cb, 0:18:17, :])
                zfill(W3["Y2"][:, cb, :, 0:18:17])
            zfill(W3["U2o"][:, 0:34:33, :], nc.vector)
            zfill(W3["U2o"][:, :, 0:34:33], nc.vector)
            zfill(W3["Ta"][:, 0:66:65, :])
            zfill(W3["Ta"][0:64, :, 0:66:65])
            zfill(W3["Ta"][64:128, :, 64:66])
            make_identity(nc, ident[:, :])
            nc.sync.dma_start(out=att_b[:, :], in_=att_d.partition_broadcast(128)[:, 0, :])
            for s, d in SETS.items():
                nb = d["bias"]
                for rep in range(128 // nb):
                    nc.sync.dma_start(out=bias_t[s][rep * nb:(rep + 1) * nb, :],
                                      in_=P[s]["b"][:, :])
                nc.sync.dma_start(out=alpha_t[s][:, :],
                                  in_=P[s]["a"].partition_broadcast(128)[:, 0, :])
            nc.sync.dma_start(out=bias_t["d2"][:, :], in_=P["d2"]["b"][0:128, :])
            nc.sync.dma_start(out=bias2_d2[:, :], in_=P["d2"]["b"][128:256, :])

            # ft_h -> F (spatial split, pad offset +1): 4 block DMAs, row-clipped
            x3 = x_d.rearrange("c (h w) -> c h w", w=128)
            for q in range(4):
                r_lo = max(0, 1 - 32 * q)
                r_hi = min(36, 129 - 32 * q)
                rows = r_hi - r_lo
                img_r0 = 32 * q + r_lo - 1
                nc.gpsimd.dma_start(
                    out=W3["F"][32 * q:32 * (q + 1), r_lo:r_hi, 1:129],
                    in_=x3[:, img_r0:img_r0 + rows, :])


            # ---------- weight synthesis ----------
            def synth(s, acc, acc_f32r):
                """Synthesize W into acc tile [Ap, nblk*(B*16)] (natural layout)."""
                d = SETS[s]
                A, B = d["A"], d["B"]
                Ap = min(A, 128)
                nblk = A // Ap
                FR = B * 16
                dma = nc.gpsimd if acc_f32r else nc.sync
                acc3 = acc.rearrange("p (blk f) -> p blk f", blk=nblk)
                for blk in range(nblk):
                    dma.dma_start(out=acc3[:, blk, :],
                                  in_=P[s]["w"][blk * Ap:(blk + 1) * Ap, :])
                for k in range(4):
                    mst = mstage_pool.tile([128, 256], f32, tag="mst")
                    m3w = mst[:Ap, :nblk * B].rearrange("p (blk b) -> p blk b", blk=nblk)
                    for blk in range(nblk):
                        nc.sync.dma_start(
                            out=m3w[:, blk, :],
                            in_=P[s]["ms"][k][blk * Ap:(blk + 1) * Ap, :])
                    m3 = mst[:Ap, :nblk * B].rearrange("p (blk b) -> p blk b", blk=nblk)
                    src_t = P[s]["ts"][k].rearrange("(blk p) f -> p blk f", blk=nblk)
                    for blk in range(nblk):
                        CH = min(1024, FR)
                        for ci0, c0 in enumerate(range(0, FR, CH)):
                            cw = min(CH, FR - c0)
                            st = stage_pool.tile([128, 1024], f32, tag="tst")
                            nc.sync.dma_start(out=st[:Ap, :cw], in_=src_t[:, blk, c0:c0 + cw])
                            tm = tmp_pool.tile([128, 1024], f32, tag="tmp")
                            nc.vector.scalar_tensor_tensor(
                                out=tm[:Ap, :cw].rearrange("p (b k) -> p b k", k=16),
                                in0=st[:Ap, :cw].rearrange("p (b k) -> p b k", k=16),
                                scalar=att_b[:Ap, k:k + 1],
                                in1=m3[:, blk, c0 // 16:(c0 + cw) // 16].unsqueeze(2)
                                    .broadcast_to([Ap, cw // 16, 16]),
                                op0=OP.mult, op1=OP.mult)
                            eng = nc.vector if (ci0 + k) % 2 == 0 else nc.gpsimd
                            eng.tensor_add(
                                out=acc3[:, blk, c0:c0 + cw],
                                in0=acc3[:, blk, c0:c0 + cw],
                                in1=tm[:Ap, :cw])

            def transpose_down(s, acc, WT, n_rep, Mt):
                """PE-transpose down weights tap by tap into WT (f32r)."""
                d = SETS[s]
                A, B = d["A"], d["B"]
                Ap = min(A, 128)
                nblk = A // Ap
                acc4 = acc.rearrange("p (blk i k) -> p blk i k", blk=nblk, k=16)
                for t in range(16):
                    for blk in range(nblk):
                        tp = tpsum_pool.tile([128, 128], f32, tag="tp")
                        nc.tensor.transpose(
                            tp[:B, :Ap], acc4[:, blk, :, t], ident[:Ap, :Ap])
                        nc.vector.tensor_copy(
                            out=WT[0:B, t * Mt + blk * 128: t * Mt + blk * 128 + Ap],
                            in_=tp[:B, :Ap])
                for rep in range(1, n_rep):
                    nc.sync.dma_start(out=WT[rep * B:(rep + 1) * B, :], in_=WT[0:B, :])

            syn_d0 = syn_pool.tile([128, 2048], f32, tag="syn")
            synth("d0", syn_d0[0:64, 0:512], False)
            transpose_down("d0", syn_d0[0:64, 0:512], W_T0, 4, 64)
            syn_d1 = syn_pool.tile([128, 2048], f32, tag="syn")
            synth("d1", syn_d1[0:128, 0:1024], False)
            transpose_down("d1", syn_d1[0:128, 0:1024], W_T1, 2, 128)
            # d2: synthesize + transpose per ob block through the syn pool
            dd = P["d2"]
            for ob in range(2):
                syn_d2 = syn_pool.tile([128, 2048], f32, tag="syn")
                nc.sync.dma_start(out=syn_d2[:, :], in_=dd["w"][ob * 128:(ob + 1) * 128, :])
                for k in range(4):
                    mst = mstage_pool.tile([128, 256], f32, tag="mst")
                    nc.sync.dma_start(out=mst[:, 0:128],
                                      in_=dd["ms"][k][ob * 128:(ob + 1) * 128, :])
                    for ci0, c0 in enumerate(range(0, 2048, 1024)):
                        st = stage_pool.tile([128, 1024], f32, tag="tst")
                        nc.sync.dma_start(out=st[:, :],
                                          in_=dd["ts"][k][ob * 128:(ob + 1) * 128, c0:c0 + 1024])
                        tm = tmp_pool.tile([128, 1024], f32, tag="tmp")
                        nc.vector.scalar_tensor_tensor(
                            out=tm[:, :].rearrange("p (b k) -> p b k", k=16),
                            in0=st[:, :].rearrange("p (b k) -> p b k", k=16),
                            scalar=att_b[:, k:k + 1],
                            in1=mst[:, c0 // 16:(c0 + 1024) // 16].unsqueeze(2)
                                .broadcast_to([128, 64, 16]),
                            op0=OP.mult, op1=OP.mult)
                        eng = nc.vector if (ci0 + k) % 2 == 0 else nc.gpsimd
                        eng.tensor_add(out=syn_d2[:, c0:c0 + 1024],
                                       in0=syn_d2[:, c0:c0 + 1024], in1=tm[:, :])
                acc4 = syn_d2[:, :].rearrange("p (i k) -> p i k", k=16)
                for t in range(16):
                    tp = tpsum_pool.tile([128, 128], f32, tag="tp")
                    nc.tensor.transpose(tp[:, :], acc4[:, :, t], ident[:, :])
                    nc.vector.tensor_copy(
                        out=W_T2[:, t * 256 + ob * 128: t * 256 + ob * 128 + 128],
                        in_=tp[:, :])

            for cb in range(2):
                nc.sync.dma_start(out=l0_t[:, cb * 256:(cb + 1) * 256],
                                  in_=l0_d[128 * cb:128 * (cb + 1), :])
            synth("u2", W_U2[:, :], True)
            synth("u1", W_U1[:, :], True)
            nc.sync.dma_start(out=l1_t[:, :], in_=l1_d[:, :])
            # u0 synthesis into padded layout: pos = (ky+1)*6 + (kx+1) within 36/o
            zfill(W_U0[:, :], nc.vector)
            u0v6 = W_U0.rearrange("p (ky o kx) -> p ky o kx", ky=6, kx=6)
            u0i = u0v6[0:64, 1:5, :, 1:5]  # interior [64, 4(ky), 32(o), 4(kx)]
            w0v = P["u0"]["w"].rearrange("p (o ky kx) -> p o ky kx", ky=4, kx=4)
            for ky in range(4):
                nc.gpsimd.dma_start(
                    out=u0v6[0:64, ky + 1, :, 1:5],
                    in_=w0v[:, :, ky, :])
            mstu = mstage_pool.tile([128, 256], f32, tag="mst")
            for k in range(4):
                nc.sync.dma_start(out=mstu[0:64, 32 * k:32 * k + 32], in_=P["u0"]["ms"][k][:, :])
            for k in range(4):
                st = stage_pool.tile([128, 1024], f32, tag="tst")
                nc.sync.dma_start(out=st[0:64, 0:512], in_=P["u0"]["ts"][k][:, :])
                tm = tmp_pool.tile([128, 1024], f32, tag="tmp")
                nc.vector.scalar_tensor_tensor(
                    out=tm[0:64, 0:512].rearrange("p (b k) -> p b k", k=16),
                    in0=st[0:64, 0:512].rearrange("p (b k) -> p b k", k=16),
                    scalar=att_b[0:64, k:k + 1],
                    in1=mstu[0:64, 32 * k:32 * k + 32].unsqueeze(2)
                        .broadcast_to([64, 32, 16]),
                    op0=OP.mult, op1=OP.mult)
                nc.vector.tensor_add(
                    out=u0i,
                    in0=u0i,
                    in1=tm[0:64, 0:512].rearrange("p (o ky kx) -> p ky o kx", ky=4, kx=4))
            # block1 = block0 shifted by +2 positions (content[p] = c0[p-2])
            nc.sync.dma_start(out=W_U0[64:128, 2:32 * 36], in_=W_U0[0:64, 0:32 * 36 - 2])
            for yh in range(2):
                nc.sync.dma_start(out=l2_t[64 * yh:64 * (yh + 1), :],
                                  in_=l2_d[:, 2048 * yh:2048 * (yh + 1)])

            # ---------- conv emitters ----------
            def emit_d0(chain):
                """F -> Y0 (chain 0,1)  or  F -> Ta with -l2 and b-dup (chain 2)."""
                Fv = W3["F"]
                for cc in range(2):
                    pss = [cpsum_pool.tile([64, 512], f32, tag="cps",
                                           name=f"psd0_{cc}_{i}") for i in range(4)]
                    for t in range(16):
                        ky, kx = t // 4, t % 4
                        for q in range(4):
                            nc.tensor.matmul(
                                pss[q][:, :],
                                W_T0[32 * q:32 * q + 32, t * 64:(t + 1) * 64],
                                Fv[32 * q:32 * q + 32,
                                   16 * cc + ky: 16 * cc + ky + 15: 2,
                                   kx: kx + 127: 2],
                                start=(t == 0), stop=(t == 15),
                                tile_position=(32 * q, 0))
                    for q in range(4):
                        y0c = 16 * q + 8 * cc
                        if chain < 2:
                            qq = q // 2
                            r0 = y0c + 1 - 32 * qq
                            nc.scalar.activation(
                                out=W3["Y0"][64 * qq:64 * qq + 64, r0:r0 + 8, 1:65],
                                in_=pss[q][:, :].rearrange("p (r c) -> p r c", c=64),
                                func=AF.Prelu, bias=bias_t["d0"][0:64, :],
                                scale=1.0, alpha=alpha_t["d0"][0:64, :])
                        else:
                            nc.scalar.activation(
                                out=pss[q][:, :], in_=pss[q][:, :],
                                func=AF.Prelu, bias=bias_t["d0"][0:64, :],
                                scale=1.0, alpha=alpha_t["d0"][0:64, :])
                            yh = y0c // 32
                            l2v = l2_t.rearrange("p (r c) -> p r c", c=64)
                            nc.vector.tensor_sub(
                                out=W3["Ta"][0:64, y0c + 1:y0c + 9, 1:65],
                                in0=pss[q][:, :].rearrange("p (r c) -> p r c", c=64),
                                in1=l2v[64 * yh:64 * yh + 64, y0c % 32:(y0c % 32) + 8, :])
                            nc.gpsimd.tensor_copy(
                                out=W3["Ta"][64:128, y0c + 1:y0c + 9, 0:64],
                                in_=W3["Ta"][0:64, y0c + 1:y0c + 9, 1:65])
                if chain < 2:
                    nc.sync.dma_start(out=W3["Y0"][64:128, 0:1, :],
                                      in_=W3["Y0"][0:64, 32:33, :])
                    nc.sync.dma_start(out=W3["Y0"][0:64, 33:36, :],
                                      in_=W3["Y0"][64:128, 1:4, :])

            def emit_d1(chain):
                """Y0 -> Y1 (chain 0) or Y1 = conv - l1 (chain 1)."""
                Yv = W3["Y0"]
                pss = [cpsum_pool.tile([128, 512], f32, tag="cps", name=f"psd1_{i}") for i in range(2)]
                for t in range(16):
                    ky, kx = t // 4, t % 4
                    for q in range(2):
                        nc.tensor.matmul(
                            pss[q][:, :],
                            W_T1[64 * q:64 * q + 64, t * 128:(t + 1) * 128],
                            Yv[64 * q:64 * q + 64, ky:ky + 31:2, kx:kx + 63:2],
                            start=(t == 0), stop=(t == 15),
                            tile_position=(64 * q, 0))
                for q in range(2):
                    y0c = 16 * q
                    if chain == 0:
                        nc.scalar.activation(
                            out=W3["Y1"][:, y0c + 1:y0c + 17, 1:33],
                            in_=pss[q][:, :].rearrange("p (r c) -> p r c", c=32),
                            func=AF.Prelu, bias=bias_t["d1"][:, :], scale=1.0,
                            alpha=alpha_t["d1"][:, :])
                    else:
                        nc.scalar.activation(
                            out=pss[q][:, :], in_=pss[q][:, :],
                            func=AF.Prelu, bias=bias_t["d1"][:, :], scale=1.0,
                            alpha=alpha_t["d1"][:, :])
                        nc.vector.tensor_sub(
                            out=W3["Y1"][:, y0c + 1:y0c + 17, 1:33],
                            in0=pss[q][:, :].rearrange("p (r c) -> p r c", c=32),
                            in1=l1_t[:, y0c * 32:(y0c + 16) * 32]
                                .rearrange("p (r c) -> p r c", c=32))

            def emit_d2():
                """Y1 -> Y2 (always -l0)."""
                Yv = W3["Y1"]
                for ob in range(2):
                    ps = cpsum_pool.tile([128, 256], f32, tag="cps")
                    for t in range(16):
                        ky, kx = t // 4, t % 4
                        nc.tensor.matmul(
                            ps[:, :],
                            W_T2[:, t * 256 + ob * 128: t * 256 + ob * 128 + 128],
                            Yv[:, ky:ky + 31:2, kx:kx + 31:2],
                            start=(t == 0), stop=(t == 15))
                    nc.scalar.activation(
                        out=ps[:, :], in_=ps[:, :],
                        func=AF.Prelu,
                        bias=bias_t["d2"][:, :] if ob == 0 else bias2_d2[:, :],
                        scale=1.0, alpha=alpha_t["d2"][:, :])
                    nc.vector.tensor_sub(
                        out=W3["Y2"][:, ob, 1:17, 1:17],
                        in0=ps[:, :].rearrange("p (r c) -> p r c", c=16),
                        in1=l0_t[:, ob * 256:(ob + 1) * 256]
                            .rearrange("p (r c) -> p r c", c=16))

            def emit_u2():
                """Y2 -> U2o."""
                Yv = W3["Y2"]
                WU = W_U2.rearrange("p (cb o k) -> p cb o k", cb=2, k=16)
                for py in range(2):
                    for px in range(2):
                        ps = cpsum_pool.tile([128, 256], f32, tag="cps")
                        first = True
                        for cb in range(2):
                            for a in range(2):
                                for b in range(2):
                                    kk = (3 - py - 2 * a) * 4 + (3 - px - 2 * b)
                                    last = (cb == 1 and a == 1 and b == 1)
                                    nc.tensor.matmul(
                                        ps[:, :],
                                        WU[:, cb, :, kk],
                                        Yv[:, cb, py + a:py + a + 16, px + b:px + b + 16],
                                        start=first, stop=last)
                                    first = False
                        nc.scalar.activation(
                            out=W3["U2o"][:, py + 1:py + 33:2, px + 1:px + 33:2],
                            in_=ps[:, :].rearrange("p (r c) -> p r c", c=16),
                            func=AF.Prelu, bias=bias_t["u2"][:, :], scale=1.0,
                            alpha=alpha_t["u2"][:, :])

            def emit_u1(src):
                """src (U2o or Y1) -> Ta (primary block0 + shifted dup block1)."""
                Yv = W3[src]
                WU = W_U1.rearrange("p (o k) -> p o k", k=16)
                for py in range(2):
                    for c in range(2):
                        for px in range(2):
                            ps = cpsum_pool.tile([64, 512], f32, tag="cps",
                                                 name=f"psu1_{py}_{c}_{px}")
                            for a in range(2):
                                for b in range(2):
                                    kk = (3 - py - 2 * a) * 4 + (3 - px - 2 * b)
                                    nc.tensor.matmul(
                                        ps[:, :],
                                        WU[:, :, kk],
                                        Yv[:, 16 * c + py + a:16 * c + py + a + 16,
                                           px + b:px + b + 32],
                                        start=(a == 0 and b == 0),
                                        stop=(a == 1 and b == 1))
                            r0 = 32 * c + py + 1
                            nc.scalar.activation(
                                out=W3["Ta"][0:64, r0:r0 + 32:2, px + 1:px + 65:2],
                                in_=ps[:, :].rearrange("p (r c) -> p r c", c=32),
                                func=AF.Prelu, bias=bias_t["u1"][0:64, :],
                                scale=1.0, alpha=alpha_t["u1"][0:64, :])
                            eng = nc.vector if px == 0 else nc.gpsimd
                            eng.tensor_copy(
                                out=W3["Ta"][64:128, r0:r0 + 32:2, px:px + 64:2],
                                in_=W3["Ta"][0:64, r0:r0 + 32:2, px + 1:px + 65:2])

            def emit_u0(chain):
                """Ta -> F += prelu(conv)  (chains 0,1) or f3 = F + prelu(conv) (2)."""
                Yv = W3["Ta"]
                WU6 = W_U0.rearrange("p (ky o kx) -> p ky o kx", ky=6, kx=6)
                for c in range(8):
                    qq = c // 2
                    for px in range(2):
                        ps = cpsum_pool.tile([64, 512], f32, tag="cps",
                                             name=f"psu0_{c}_{px}")
                        for r in range(3):
                            nc.tensor.matmul(
                                ps[:, :],
                                WU6[:, 4 - 2 * r:6 - 2 * r, :, 4 - px],
                                Yv[:, 8 * c + r:8 * c + r + 8, px:px + 64],
                                start=(r == 0), stop=(r == 2))
                        nc.scalar.activation(
                            out=ps[:, :], in_=ps[:, :],
                            func=AF.Prelu, bias=bias_t["u0"][0:64, :], scale=1.0,
                            alpha=alpha_t["u0"][0:64, :])
                        for py in range(2):
                            r0 = 16 * (c % 2) + py + 1
                            s3 = ps[32 * py:32 * py + 32, :].rearrange(
                                "p (r c) -> p r c", c=64)
                            if chain < 2:
                                nc_sl = W3["F"][32 * qq:32 * qq + 32,
                                                r0:r0 + 16:2, px + 1:px + 129:2]
                                nc.vector.tensor_add(out=nc_sl, in0=nc_sl, in1=s3)
                            else:
                                rr = 16 * (c % 2) + py
                                nc.vector.tensor_add(
                                    out=W3["f3"][32 * qq:32 * qq + 32, rr:rr + 15:2,
                                                 px:px + 127:2],
                                    in0=W3["F"][32 * qq:32 * qq + 32, r0:r0 + 16:2,
                                                px + 1:px + 129:2],
                                    in1=s3)
                if chain < 2:
                    for q in range(1, 4):
                        nc.sync.dma_start(out=W3["F"][32 * q:32 * q + 32, 0:1, :],
                                          in_=W3["F"][32 * (q - 1):32 * q, 32:33, :])
                    for q in range(3):
                        nc.sync.dma_start(out=W3["F"][32 * q:32 * q + 32, 33:36, :],
                                          in_=W3["F"][32 * (q + 1):32 * (q + 2), 1:4, :])

            # ---------- the three chains ----------
            emit_d0(0); emit_d1(0); emit_d2(); emit_u2(); emit_u1("U2o"); emit_u0(0)
            emit_d0(1); emit_d1(1); emit_u1("Y1"); emit_u0(1)
            emit_d0(2); emit_u0(2)

            for yq in range(4):
                nc.sync.dma_start(out=out_d[:, yq * 4096:(yq + 1) * 4096],
                                  in_=f3[32 * yq:32 * (yq + 1), :])

            if debug:
                nc.sync.dma_start(out=dbg["dY0"][:, :], in_=Y0[:, :])
                nc.sync.dma_start(out=dbg["dY1"][:, :], in_=Y1[:, :])
                nc.sync.dma_start(out=dbg["dY2"][:, :], in_=Y2[:, :])
                nc.sync.dma_start(out=dbg["dU2o"][:, :], in_=U2o[:, :])
                nc.sync.dma_start(out=dbg["dTa"][:, :], in_=Ta[:, :])
                nc.sync.dma_start(out=dbg["dF"][:, :], in_=F[:, :])

    nc.compile()
    return nc


def _get_nc(debug=False):
    key = bool(debug)
    if key not in _BUILT:
        _BUILT[key] = _build(debug)
    return _BUILT[key]


def make_in_maps(ft_h, ft_l0, ft_l1, ft_l2, att, params):
    ft_h = np.asarray(ft_h, dtype=np.float32)
    ft_l0 = np.asarray(ft_l0, dtype=np.float32)
    ft_l1 = np.asarray(ft_l1, dtype=np.float32)
    ft_l2 = np.asarray(ft_l2, dtype=np.float32)
    att = np.asarray(att, dtype=np.float32)
    B = ft_h.shape[0]
    base = {}
    for grp, pref in [("down", "d"), ("up", "u")]:
        for j in range(3):
            p = params[grp][j]
            s = f"{pref}{j}"
            w = np.asarray(p["w"], np.float32)
            A, Bd = w.shape[0], w.shape[1]
            base[f"{s}_w"] = np.ascontiguousarray(w.reshape(A, Bd * 16))
            for k, key in enumerate(["tb", "tq", "tn", "tx"]):
                base[f"{s}_t{k}"] = np.ascontiguousarray(
                    np.asarray(p[key], np.float32).reshape(A, Bd * 16))
            for k, key in enumerate(["mb", "mq", "mn", "mx"]):
                base[f"{s}_m{k}"] = np.ascontiguousarray(
                    np.asarray(p[key], np.float32).reshape(A, Bd))
            base[f"{s}_b"] = np.ascontiguousarray(
                np.asarray(p["b"], np.float32).reshape(-1, 1))
            base[f"{s}_a"] = np.asarray(p["a"], np.float32).reshape(1, 1)
    in_maps = []
    for b in range(B):
        m = dict(base)
        m["x"] = np.ascontiguousarray(ft_h[b].reshape(32, 128 * 128))
        m["l0"] = np.ascontiguousarray(ft_l0[b].reshape(256, 256))
        m["l1"] = np.ascontiguousarray(ft_l1[b].reshape(128, 1024))
        m["l2"] = np.ascontiguousarray(ft_l2[b].reshape(64, 4096))
        m["att"] = np.ascontiguousarray(att[b:b + 1, :])
        in_maps.append(m)
    return in_maps


def run(ft_h, ft_l0, ft_l1, ft_l2, att, params, debug=False, trace=False):
    from concourse import bass_utils
    nc = _get_nc(debug)
    in_maps = make_in_maps(ft_h, ft_l0, ft_l1, ft_l2, att, params)
    res = bass_utils.run_bass_kernel_spmd(
        nc, in_maps, core_ids=list(range(NCORES)), trace=trace)
    outs = np.stack([r["out"].reshape(32, 128, 128) for r in res.results])
    return outs, res


def kernel(ft_h, ft_l0, ft_l1, ft_l2, att, params):
    outs, _ = run(ft_h, ft_l0, ft_l1, ft_l2, att, params)
    return outs
